# revision 60
# baseline (speedup 1.0000x reference)
"""Multi-head self-attention (B=8, S=1024, E=768, H=12, D=64) on 8 NeuronCores.

Sharding: data-parallel over batch — one batch element per core, weights
replicated, no collectives.

Per-core dataflow (layouts chosen so the only transpose is x -> xT):
  1. xT = x^T via PE transpose (48 128x128 tiles), cast to fp32r on copy-out.
  2. QT[e',s] = (x Wq + bq)^T per head-pair: lhsT=Wq tile, rhs=xT.
  3. V scattered into V_ext[s, ktile, head, 128] = [V_h+bv | ones] (even head)
     or [ones | V_h+bv] (odd head).  Adding bv here is exact: softmax rows
     sum to 1, so attn + bv == (sum_k E_k (V_k + bv)) / sums.
  4. Per pair, per q-tile: scores^T[k,q] = KT.T @ QT (row pairs at partition
     base 0/64), exp on ACT with the 1/sqrt(D)=1/8 scale folded in (no max
     subtraction needed: scores ~ N(0,1)).
  5. attnV: one M=128 matmul per (head, ktile) -> rows [attn^T|sums] (even)
     / [sums|attn^T] (odd).  One reciprocal + aligned multiplies normalize.
  6. out = concatT.T @ Wo + bo (bo broadcast via partition-step-0 DMA).

Matmuls run in float32r (TF32-rate fp32): operand tiles are dtype float32r,
produced by DVE/ACT cast-on-write (walrus requires fp32r operands to be
written pre-rounded).  The attention-value path (V_ext, exp output E) is
bf16 for SBUF footprint; accumulation stays fp32 in PSUM.

Measured on trn2 (8 cores): HW exec ~270us, rel-rms error 1.75e-3,
absmax/scale 1.5e-2 vs the fp32 jax reference.

Notes on two hardware workarounds baked in here:
 - This walrus build rejects instructions carrying more than ~1-2 sync
   waits ("Too many sync wait commands"); _split_excess_waits and the
   patched TileContext tail hoist surplus waits onto standalone EVSEM ops.
 - DVE reads from PSUM with a partition-base offset different from the
   output's silently return wrong data (measured), so the softmax-sums
   half-swap goes through partition-aligned copies + an SBUF->SBUF DMA
   (crossbase=False).  Do not enable crossbase.
"""
import sys
sys.path.insert(0, "/opt/trn_rl_repo")
from contextlib import ExitStack

import numpy as np

import concourse.bass as bass
import concourse.tile as tile
from concourse import mybir
from concourse.bass_utils import run_bass_kernel_spmd
from concourse.vector_clock import ScopedClock


def _split_drain_and_barrier(self, tick_clock, wait_clock):
    """TileContext tail with the final drain's waits split one-per-instruction.

    The stock tail puts every pending processor tick (engines + up to 16 DMA
    queue sems) as waits on a single InstDrain; walrus codegen caps non-EVSEM
    instructions at one sync wait and fails with 'Too many sync wait
    commands'.  Emit a bare drain followed by single-wait EVSEM wait_ge ops
    instead.
    """
    drain_inst = self.nc.sync.drain()
    wait_clock.add_sem_waits(
        drain_inst.ins, ScopedClock({None: tick_clock.global_clock})
    )
    si = drain_inst.ins.sync_info
    waits = list(si.on_wait) if si is not None and si.on_wait else []
    if len(waits) > 1:
        si.on_wait = []
        by_num = {h.num: h for h in self.sems.allocated().values()}
        for w in waits:
            self.nc.sync.wait_ge(by_num[w.id], w.wait_value)
    self.nc.all_engine_barrier()
    popped = self.nc._tile_sem_poison_stack.pop()
    assert popped is self._sem_poison
    self.nc.clear_and_free_semaphores(list(self.sems.allocated().values()))
    self.nc.all_engine_barrier()


tile.TileContext._drain_and_barrier = _split_drain_and_barrier


def _split_excess_waits(nc):
    """Hoist excess per-instruction sync waits into standalone EVSEM waits.

    This walrus build caps sync-wait commands per instruction (1 for most
    structs, 2 for EventSemaphore); Tile's wait pass can attach more.  Move
    the surplus onto fresh single-wait InstEventSemaphore ops on the same
    engine, placed immediately before the owning instruction — identical
    stall semantics, codegen-legal.
    """
    counter = 0
    for f in nc.m.functions:
        for bb in f.blocks:
            insts = bb.instructions
            out = []
            for inst in insts:
                si = inst.sync_info
                cap = 2 if isinstance(inst, mybir.InstEventSemaphore) else 1
                if si is not None and si.on_wait and len(si.on_wait) > cap:
                    waits = list(si.on_wait)
                    for w in waits[cap:]:
                        counter += 1
                        ev = mybir.InstEventSemaphore(name=f"I-wsplit-{counter}")
                        ev.engine = inst.engine
                        ev.sync_info = mybir.SyncInfo(on_wait=[w], on_update=[])
                        out.append(ev)
                    si.on_wait = waits[:cap]
                out.append(inst)
            if len(out) != len(insts):
                insts[:] = out
    return counter

P = 128
S = 1024
E = 768
H = 12
D = 64
KT = E // P        # 6 e-tiles
ST = S // P        # 8 s-tiles
NPAIR = H // 2     # 6 head pairs
QTILE = 512
NQ = S // QTILE    # 2 q-tiles
ESLICES = [(0, 512), (512, 256)]

f32 = mybir.dt.float32
f32r = mybir.dt.float32r
bf16 = mybir.dt.bfloat16
EXP = mybir.ActivationFunctionType.Exp
LN = mybir.ActivationFunctionType.Ln

_NC_CACHE = {}


def build(mm_dtype="bf16", e_dtype="bf16", crossbase=False):
    mdt = {"f32r": f32r, "f32": f32, "bf16": bf16}[mm_dtype]
    edt = {"f32r": f32r, "f32": f32, "bf16": bf16}[e_dtype]
    # The host pre-lays-out every tensor in its SBUF tile geometry (and
    # pre-casts to bf16 in bf16 mode), so every DMA is a contiguous burst:
    #   x  -> x^T as [P, KT, S]   (kills the on-device transpose entirely)
    #   Wq/Wk -> [NPAIR, P, KT, P] head-pair-major
    #   Wv/Wo -> [P, KT, E]
    #   bq/bk -> [P, KT]
    wdt = bf16 if mm_dtype == "bf16" else f32
    nc = bass.Bass()
    x_d = nc.declare_dram_parameter("x", [P, KT, S], wdt, isOutput=False)
    Wq_d = nc.declare_dram_parameter("Wq", [NPAIR, P, KT, P], wdt, isOutput=False)
    Wk_d = nc.declare_dram_parameter("Wk", [NPAIR, P, KT, P], wdt, isOutput=False)
    Wv_d = nc.declare_dram_parameter("Wv", [P, KT, E], wdt, isOutput=False)
    Wo_d = nc.declare_dram_parameter("Wo", [P, KT, E], wdt, isOutput=False)
    bq_d = nc.declare_dram_parameter("bq", [P, KT], f32, isOutput=False)
    bk_d = nc.declare_dram_parameter("bk", [P, KT], f32, isOutput=False)
    bv_d = nc.declare_dram_parameter("bv", [E], f32, isOutput=False)
    bo_d = nc.declare_dram_parameter("bo", [E], f32, isOutput=False)
    # bf16 output (host casts back to f32): halves the output DMA traffic
    out_d = nc.declare_dram_parameter("out", [S, E], wdt, isOutput=True)

    with ExitStack() as ctx:
        tc = ctx.enter_context(tile.TileContext(nc))
        singles = ctx.enter_context(tc.tile_pool(name="singles", bufs=1))
        xld = ctx.enter_context(tc.tile_pool(name="xld", bufs=2))
        wqk = ctx.enter_context(tc.tile_pool(name="wqk", bufs=2))
        wbig = ctx.enter_context(tc.tile_pool(name="wbig", bufs=1))
        qkp = ctx.enter_context(tc.tile_pool(name="qkp", bufs=2))
        ep = ctx.enter_context(tc.tile_pool(name="ep", bufs=2))
        np_pool = ctx.enter_context(tc.tile_pool(name="norm", bufs=2))
        outp = ctx.enter_context(tc.tile_pool(name="outp", bufs=2))
        # PSUM: mm(2) + S(2x2) + att(2) = 8 banks
        psum = ctx.enter_context(tc.tile_pool(name="psum", bufs=2, space="PSUM"))

        # ---- persistent big buffers ----
        xT = singles.tile([P, KT, S], mdt)          # x^T  [e_in, s]
        V_ext = singles.tile([P, ST, H, P], edt)    # [s, ktile, head, ...]
        concatT = singles.tile([P, NPAIR, S], mdt)  # attn^T by pair

        # ---- phase 1: load x^T (emitted first: it gates everything) ----
        # Three hardware DMA queues in parallel: x^T halves on the sync
        # queue, the pair-0-critical weights (Wv, Wq0/Wk0) on the scalar
        # HWDGE queue (ACT is idle early), the rest on the gpsimd queue.
        def xt_load(dst, src, eng):
            if mdt != f32r:
                eng.dma_start(dst, src)
            else:
                stg = xld.tile([P, S], f32, tag="x", name="xstage")
                nc.sync.dma_start(stg[:], src.rearrange("p k s -> p (k s)"))
                nc.vector.tensor_copy(
                    dst, stg[:].rearrange("p (k s) -> p k s", s=S))

        # per-k chunks split across two queues so the first projection's
        # k-accumulation chain streams right behind the DMAs (scalar queue
        # carries wq0/wk0 — they gate the first matmul — then Wv)
        for k in range(KT):
            xt_load(xT[:, k:k + 1, :], x_d[:, k:k + 1, :],
                    nc.sync if k < 3 else nc.gpsimd)

        def wload(dst_r, src_slice_ap, eng=None):
            """Load weights into an SBUF tile of matmul dtype.

            bf16/f32 mode: DRAM layout already matches, one direct DMA on
            the given DMA queue (default gpsimd).
            f32r mode: DMA f32 to a staging tile, DVE-cast into fp32r dst.
            """
            if mdt != f32r:
                (eng or nc.gpsimd).dma_start(dst_r[:], src_slice_ap)
            else:
                stg = wqk.tile(list(dst_r.shape), f32, tag="wstg", name="wstg")
                nc.sync.dma_start(stg[:], src_slice_ap)
                nc.vector.tensor_copy(dst_r[:], stg[:])

        # ---- constants ----
        bq_sb = singles.tile([P, KT], f32)
        bk_sb = singles.tile([P, KT], f32)
        nc.sync.dma_start(bq_sb[:], bq_d[:])
        nc.sync.dma_start(bk_sb[:], bk_d[:])

        def bcast_load(dst, src_ap):  # [E] -> [P, E] partition-step-0 DMA
            nc.gpsimd.dma_start(
                out=dst,
                in_=bass.AP(tensor=src_ap.tensor, offset=src_ap.offset,
                            ap=[[0, P]] + [list(a) for a in src_ap.ap]))
        bcast = ctx.enter_context(tc.tile_pool(name="bcast", bufs=1))
        bv_bc = bcast.tile([P, E], f32, tag="bbc")
        bcast_load(bv_bc[:], bv_d[:])

        # Only the ones-halves need the memset (the V projection writes the
        # V halves); split across DVE+GPSIMD so it clears in ~3us.
        V5 = V_ext[:].rearrange("p s (j par) c -> p s j par c", par=2)
        nc.vector.memset(V5[:, :, :, 0, D:P], 1.0)
        nc.gpsimd.memset(V5[:, :, :, 1, 0:D], 1.0)

        # ---- phase 2: V projection into V_ext ----
        # Emitted between the first scores and first attnV, so these matmuls
        # fill the PE while ACT computes the first exps.  PSUM evacuation is
        # batched per parity (strided APs over 4/2 heads at once) so the two
        # mm PSUM banks recycle fast enough to keep the PE streaming.
        def v_proj_all():
            for st in range(ST):
                Vx = V_ext[:, st, :, :].rearrange("p (j par) c -> p j par c",
                                                  par=2)
                for nsi, (noff, nsz) in enumerate(ESLICES):
                    pv = psum.tile([P, 512], f32, tag="mm")
                    for k in range(KT):
                        nc.tensor.matmul(
                            pv[:, :nsz],
                            xT[:, k, st * P:(st + 1) * P],
                            Wv_sb[:, k, noff:noff + nsz],
                            start=(k == 0), stop=(k == KT - 1),
                        )
                    j0, nj = noff // P, nsz // P
                    pv_v = pv[:, :nsz].rearrange("p (j par d) -> p j par d",
                                                 par=2, d=D)
                    bv_v = bv_bc[:, noff:noff + nsz].rearrange(
                        "p (j par d) -> p j par d", par=2, d=D)
                    nc.vector.tensor_add(Vx[:, j0:j0 + nj, 0, 0:D],
                                         pv_v[:, :, 0, :], bv_v[:, :, 0, :])
                    nc.vector.tensor_add(Vx[:, j0:j0 + nj, 1, D:P],
                                         pv_v[:, :, 1, :], bv_v[:, :, 1, :])

        # ---- phase 3: head pairs, software-pipelined ----
        # PE order per (pair, q-half): scores_m -> next pair's Q or K
        # projection matmuls -> attnV_m.  The projection matmuls fill the PE
        # while ACT computes this iteration's exps, keeping the PE dense (HAM
        # stays at full clock) instead of stalling on E.  The projections'
        # DVE bias-adds are deferred until after attnV's PSUM evacuation so
        # the att banks recycle first in the DVE queue.
        wq_t, wk_t, qt_t, kt_t = {}, {}, {}, {}
        def load_w(m, eng=None):
            wq_t[m] = wqk.tile([P, KT, P], mdt, tag="wq", name="wq_m")
            wk_t[m] = wqk.tile([P, KT, P], mdt, tag="wk", name="wk_m")
            wload(wq_t[m], Wq_d[m], eng=eng)
            wload(wk_t[m], Wk_d[m], eng=eng)

        # Scalar-queue order: pair-0 weights first (they gate the first
        # matmul), then Wv (first needed ~10us in).  Pair-1 and Wo/bo go on
        # the gpsimd queue.
        load_w(0, eng=nc.scalar)
        Wv_sb = wbig.tile([P, KT, E], mdt, tag="wbig")
        wload(Wv_sb, Wv_d[:], eng=nc.scalar)
        load_w(1)
        bo_bc = bcast.tile([P, E], f32, tag="bbc")
        bcast_load(bo_bc[:], bo_d[:])
        Wo_sb = wbig.tile([P, KT, E], mdt, tag="wbig")
        wload(Wo_sb, Wo_d[:])

        def proj_mm(m, which, q2):
            """6 matmuls: one q-half of QT_m (or KT_m) into a PSUM tile."""
            w = wq_t[m] if which == "q" else wk_t[m]
            tmap = qt_t if which == "q" else kt_t
            if m not in tmap:
                tmap[m] = qkp.tile([P, S], mdt, tag=which + "t", name=which + "t")
            qsl = slice(q2 * QTILE, (q2 + 1) * QTILE)
            pq = psum.tile([P, 512], f32, tag="mm", name="pq")
            for k in range(KT):
                nc.tensor.matmul(pq[:], w[:, k, :], xT[:, k, qsl],
                                 start=(k == 0), stop=(k == KT - 1))
            return pq

        def proj_fin(m, which, q2, pq):
            """Deferred DVE bias-add: PSUM -> QT/KT tile."""
            bias = bq_sb if which == "q" else bk_sb
            t = (qt_t if which == "q" else kt_t)[m]
            qsl = slice(q2 * QTILE, (q2 + 1) * QTILE)
            nc.vector.tensor_scalar_add(t[:, qsl], pq[:], bias[:, m:m + 1])

        def proj_half(m, which, q2):
            proj_fin(m, which, q2, proj_mm(m, which, q2))

        def scores_exp(m, q2):
            """Score matmuls + exp for both heads of pair m, one q-half."""
            qt_m, kt_m = qt_t[m], kt_t[m]
            qsl = slice(q2 * QTILE, (q2 + 1) * QTILE)
            e_a = ep.tile([P, ST, QTILE], edt, tag="eA")
            e_b = ep.tile([P, ST, QTILE], edt, tag="eB")
            for c in range(ST // 2):
                s_a = psum.tile([P, 2, 512], f32, tag="S")
                s_b = psum.tile([P, 2, 512], f32, tag="S")
                for kk in range(2):
                    ktile = c * 2 + kk
                    ksl = slice(ktile * P, (ktile + 1) * P)
                    nc.tensor.matmul(s_a[:, kk, :], kt_m[0:D, ksl],
                                     qt_m[0:D, qsl], start=True, stop=True)
                    nc.tensor.matmul(s_b[:, kk, :], kt_m[D:P, ksl],
                                     qt_m[D:P, qsl], start=True, stop=True)
                nc.scalar.activation(e_a[:, c * 2:c * 2 + 2, :], s_a[:], EXP, scale=0.125)
                nc.scalar.activation(e_b[:, c * 2:c * 2 + 2, :], s_b[:], EXP, scale=0.125)
            return e_a, e_b

        def attnv_norm(m, q2, e_a, e_b):
            """attnV matmuls + softmax normalization, one q-half."""
            qsl = slice(q2 * QTILE, (q2 + 1) * QTILE)
            # attnV: rows [attn|sums] (even head) / [sums|attn] (odd head)
            p_a = psum.tile([P, 512], f32, tag="att")
            p_b = psum.tile([P, 512], f32, tag="att")
            for ktile in range(ST):
                nc.tensor.matmul(p_a[:], V_ext[:, ktile, 2 * m, :],
                                 e_a[:, ktile, :],
                                 start=(ktile == 0), stop=(ktile == ST - 1))
            for ktile in range(ST):
                nc.tensor.matmul(p_b[:], V_ext[:, ktile, 2 * m + 1, :],
                                 e_b[:, ktile, :],
                                 start=(ktile == 0), stop=(ktile == ST - 1))
            # sums half-swap: partition-aligned DVE copies (crossbase DVE
            # PSUM reads are broken, see module docstring) + SBUF DMA; the
            # reciprocal is exp(-ln(sums)) on ACT (~3x faster than DVE
            # InstReciprocal), then DVE multiplies straight out of the att
            # banks into concatT.
            sums_t = np_pool.tile([P, 512], f32, tag="sums_t")
            attv = np_pool.tile([P, 512], f32, tag="attv")
            nc.vector.tensor_copy(sums_t[D:P, :], p_a[D:P, :])
            nc.vector.tensor_copy(attv[0:D, :], p_a[0:D, :])
            nc.vector.tensor_copy(sums_t[0:D, :], p_b[0:D, :])
            nc.vector.tensor_copy(attv[D:P, :], p_b[D:P, :])
            sums = np_pool.tile([P, 512], f32, tag="sums")
            nc.sync.dma_start(sums[0:D, :], sums_t[D:P, :])
            nc.sync.dma_start(sums[D:P, :], sums_t[0:D, :])
            return sums, attv

        def norm_fin(m, q2, sums, attv):
            # 1/sums as exp(-ln(sums)) on ACT: ~3x faster than the DVE
            # InstReciprocal and keeps the DVE queue free for evacuations.
            # Pair 0 uses the DVE reciprocal instead: during warmup ACT is
            # the pipeline-fill bottleneck (first exps) while DVE has slack.
            qsl = slice(q2 * QTILE, (q2 + 1) * QTILE)
            if m == 0:
                nc.vector.reciprocal(sums[:], sums[:])
            else:
                lns = np_pool.tile([P, 512], f32, tag="lns")
                nc.scalar.activation(lns[:], sums[:], LN)
                nc.scalar.activation(sums[:], lns[:], EXP, scale=-1.0)
            nc.vector.tensor_mul(concatT[:, m, qsl], attv[:], sums[:])

        def out_proj(st, k_hi=KT, pos=None, tag="mm"):
            """Output projection for one s-tile.

            k_hi < KT emits a partial accumulation (pairs 0..k_hi-1) and
            returns the open PSUM groups; call again with pos=... to add the
            remaining pairs, close the groups, and store.  tag="S" borrows
            the (free, post-scores) S-tile banks for extra partials.
            """
            pos = pos or {}
            k_lo = pos.pop("k_lo", 0)
            for nsi, (noff, nsz) in enumerate(ESLICES):
                po = pos.get(nsi)
                if po is None:
                    if tag == "S":
                        s_po = psum.tile([P, 2, 512], f32, tag="S", name="po")
                        po = s_po[:, nsi, :]
                        if nsi == 0:
                            pos["s_tile"] = s_po
                    else:
                        po = psum.tile([P, 512], f32, tag="mm", name="po")[:]
                    pos[nsi] = po
                for k in range(k_lo, k_hi):
                    nc.tensor.matmul(
                        po[:, :nsz],
                        concatT[:, k, st * P:(st + 1) * P],
                        Wo_sb[:, k, noff:noff + nsz],
                        start=(k == 0), stop=(k == KT - 1),
                    )
            if k_hi < KT:
                pos["k_lo"] = k_hi
                return pos
            o_sb = outp.tile([P, E], wdt, tag="o")
            oq = nc.sync if st % 2 == 0 else nc.scalar
            for nsi, (noff, nsz) in enumerate(ESLICES):
                po = pos[nsi]
                nc.vector.tensor_add(o_sb[:, noff:noff + nsz], po[:, :nsz],
                                     bo_bc[:, noff:noff + nsz])
                oq.dma_start(out_d[st * P:(st + 1) * P, noff:noff + nsz],
                             o_sb[:, noff:noff + nsz])
            return None

        # Priming: only the pair-0 projections the first scores needs (qt
        # half-1 is deferred past them), first scores, then qt half-1 and
        # the whole V projection as the PE filler under the first exps.
        proj_half(0, "q", 0)
        proj_half(0, "k", 0)
        proj_half(0, "k", 1)
        e_pend = scores_exp(0, 0)
        proj_half(0, "q", 1)
        v_proj_all()
        for m in range(NPAIR):
            last = m + 1 == NPAIR
            if m >= 1 and not last:
                load_w(m + 1)
            for q2 in range(NQ):
                ea, eb = e_pend if (m, q2) == (0, 0) else scores_exp(m, q2)
                # Fill the PE while ACT computes this iteration's exps:
                # next pair's projection matmuls (bias-adds deferred so the
                # attnV evacuation copies go first in the DVE queue), or on
                # the last pair the first half of the output projection,
                # which only needs q2=0 of concatT.  Slot (0,0) is filled by
                # the V projection above; slot (0,1) carries pair-1's q and
                # k projections (q with inline fins — only two mm PSUM bufs).
                pqs = []
                if (m, q2) == (0, 0):
                    pass
                elif (m, q2) == (0, 1):
                    proj_half(1, "q", 0)
                    proj_half(1, "q", 1)
                    pqs = [("k", h2, proj_mm(1, "k", h2)) for h2 in range(NQ)]
                elif not last:
                    wh = "q" if q2 == 0 else "k"
                    pqs = [(wh, h2, proj_mm(m + 1, wh, h2)) for h2 in range(NQ)]
                elif q2 == 1:
                    for st in range(ST // 2):
                        out_proj(st)
                sums, attv = attnv_norm(m, q2, ea, eb)
                if last and q2 == 1:
                    # fill the norm-chain wait: st 4..6 over pairs 0..4
                    # (pair 5 accumulates in the finishers below once its
                    # concatT lands); st 5/6 borrow the freed S banks
                    parts = [out_proj(4, k_hi=KT - 1),
                             out_proj(5, k_hi=KT - 1, tag="S"),
                             out_proj(6, k_hi=KT - 1, tag="S")]
                for wh, h2, pq in pqs:
                    proj_fin(m + 1, wh, h2, pq)
                norm_fin(m, q2, sums, attv)
        # ---- phase 4: output projection (second half) ----
        for st, pos in zip(range(4, ST), parts + [None]):
            out_proj(st, pos=pos)

    _split_excess_waits(nc)
    return nc


def run_spmd(inputs, Wq, bq, Wk, bk, Wv, bv, Wo, bo,
             mm_dtype="bf16", e_dtype="bf16", crossbase=False, trace=False):
    key = (mm_dtype, e_dtype, crossbase)
    if key not in _NC_CACHE:
        _NC_CACHE[key] = build(mm_dtype, e_dtype, crossbase)
    nc = _NC_CACHE[key]
    if mm_dtype == "bf16":
        import ml_dtypes
        wnp = ml_dtypes.bfloat16
    else:
        wnp = np.float32
    # Host-side layout prep: every tensor lands in its SBUF tile geometry so
    # every device DMA is a contiguous burst (see build()).
    x = np.asarray(inputs, dtype=np.float32)
    Wq_h = (np.asarray(Wq, np.float32).reshape(KT, P, NPAIR, P)
            .transpose(2, 1, 0, 3).astype(wnp))
    Wk_h = (np.asarray(Wk, np.float32).reshape(KT, P, NPAIR, P)
            .transpose(2, 1, 0, 3).astype(wnp))
    Wv_h = (np.asarray(Wv, np.float32).reshape(KT, P, E)
            .transpose(1, 0, 2).astype(wnp))
    Wo_h = (np.asarray(Wo, np.float32).reshape(KT, P, E)
            .transpose(1, 0, 2).astype(wnp))
    common = {
        "Wq": Wq_h, "Wk": Wk_h, "Wv": Wv_h, "Wo": Wo_h,
        "bq": np.ascontiguousarray(np.asarray(bq, np.float32).reshape(KT, P).T),
        "bk": np.ascontiguousarray(np.asarray(bk, np.float32).reshape(KT, P).T),
        "bv": np.asarray(bv, np.float32), "bo": np.asarray(bo, np.float32),
    }
    in_maps = [
        dict(common,
             x=x[b].T.reshape(KT, P, S).transpose(1, 0, 2).astype(wnp))
        for b in range(x.shape[0])
    ]
    res = run_bass_kernel_spmd(nc, in_maps, core_ids=list(range(len(in_maps))),
                               trace=trace)
    out = np.stack([res.results[b]["out"] for b in range(len(in_maps))],
                   axis=0).astype(np.float32)
    return out, res


def kernel(inputs, Wq, bq, Wk, bk, Wv, bv, Wo, bo):
    out, _ = run_spmd(inputs, Wq, bq, Wk, bk, Wv, bv, Wo, bo)
    return out



# revision 62
# speedup vs baseline: 1.0228x; 1.0228x over previous
"""Multi-head self-attention (B=8, S=1024, E=768, H=12, D=64) on 8 NeuronCores.

Sharding: data-parallel over batch — one batch element per core, weights
replicated, no collectives.

Per-core dataflow (layouts chosen so the only transpose is x -> xT):
  1. xT = x^T via PE transpose (48 128x128 tiles), cast to fp32r on copy-out.
  2. QT[e',s] = (x Wq + bq)^T per head-pair: lhsT=Wq tile, rhs=xT.
  3. V scattered into V_ext[s, ktile, head, 128] = [V_h+bv | ones] (even head)
     or [ones | V_h+bv] (odd head).  Adding bv here is exact: softmax rows
     sum to 1, so attn + bv == (sum_k E_k (V_k + bv)) / sums.
  4. Per pair, per q-tile: scores^T[k,q] = KT.T @ QT (row pairs at partition
     base 0/64), exp on ACT with the 1/sqrt(D)=1/8 scale folded in (no max
     subtraction needed: scores ~ N(0,1)).
  5. attnV: one M=128 matmul per (head, ktile) -> rows [attn^T|sums] (even)
     / [sums|attn^T] (odd).  One reciprocal + aligned multiplies normalize.
  6. out = concatT.T @ Wo + bo (bo broadcast via partition-step-0 DMA).

Matmuls run in float32r (TF32-rate fp32): operand tiles are dtype float32r,
produced by DVE/ACT cast-on-write (walrus requires fp32r operands to be
written pre-rounded).  The attention-value path (V_ext, exp output E) is
bf16 for SBUF footprint; accumulation stays fp32 in PSUM.

Measured on trn2 (8 cores): HW exec ~270us, rel-rms error 1.75e-3,
absmax/scale 1.5e-2 vs the fp32 jax reference.

Notes on two hardware workarounds baked in here:
 - This walrus build rejects instructions carrying more than ~1-2 sync
   waits ("Too many sync wait commands"); _split_excess_waits and the
   patched TileContext tail hoist surplus waits onto standalone EVSEM ops.
 - DVE reads from PSUM with a partition-base offset different from the
   output's silently return wrong data (measured), so the softmax-sums
   half-swap goes through partition-aligned copies + an SBUF->SBUF DMA
   (crossbase=False).  Do not enable crossbase.
"""
import sys
sys.path.insert(0, "/opt/trn_rl_repo")
from contextlib import ExitStack

import numpy as np

import concourse.bass as bass
import concourse.tile as tile
from concourse import mybir
from concourse.bass_utils import run_bass_kernel_spmd
from concourse.vector_clock import ScopedClock


def _split_drain_and_barrier(self, tick_clock, wait_clock):
    """TileContext tail with the final drain's waits split one-per-instruction.

    The stock tail puts every pending processor tick (engines + up to 16 DMA
    queue sems) as waits on a single InstDrain; walrus codegen caps non-EVSEM
    instructions at one sync wait and fails with 'Too many sync wait
    commands'.  Emit a bare drain followed by single-wait EVSEM wait_ge ops
    instead.
    """
    drain_inst = self.nc.sync.drain()
    wait_clock.add_sem_waits(
        drain_inst.ins, ScopedClock({None: tick_clock.global_clock})
    )
    si = drain_inst.ins.sync_info
    waits = list(si.on_wait) if si is not None and si.on_wait else []
    if len(waits) > 1:
        si.on_wait = []
        by_num = {h.num: h for h in self.sems.allocated().values()}
        for w in waits:
            self.nc.sync.wait_ge(by_num[w.id], w.wait_value)
    self.nc.all_engine_barrier()
    popped = self.nc._tile_sem_poison_stack.pop()
    assert popped is self._sem_poison
    self.nc.clear_and_free_semaphores(list(self.sems.allocated().values()))
    self.nc.all_engine_barrier()


tile.TileContext._drain_and_barrier = _split_drain_and_barrier


def _split_excess_waits(nc):
    """Hoist excess per-instruction sync waits into standalone EVSEM waits.

    This walrus build caps sync-wait commands per instruction (1 for most
    structs, 2 for EventSemaphore); Tile's wait pass can attach more.  Move
    the surplus onto fresh single-wait InstEventSemaphore ops on the same
    engine, placed immediately before the owning instruction — identical
    stall semantics, codegen-legal.
    """
    counter = 0
    for f in nc.m.functions:
        for bb in f.blocks:
            insts = bb.instructions
            out = []
            for inst in insts:
                si = inst.sync_info
                cap = 2 if isinstance(inst, mybir.InstEventSemaphore) else 1
                if si is not None and si.on_wait and len(si.on_wait) > cap:
                    waits = list(si.on_wait)
                    for w in waits[cap:]:
                        counter += 1
                        ev = mybir.InstEventSemaphore(name=f"I-wsplit-{counter}")
                        ev.engine = inst.engine
                        ev.sync_info = mybir.SyncInfo(on_wait=[w], on_update=[])
                        out.append(ev)
                    si.on_wait = waits[:cap]
                out.append(inst)
            if len(out) != len(insts):
                insts[:] = out
    return counter

P = 128
S = 1024
E = 768
H = 12
D = 64
KT = E // P        # 6 e-tiles
ST = S // P        # 8 s-tiles
NPAIR = H // 2     # 6 head pairs
QTILE = 512
NQ = S // QTILE    # 2 q-tiles
ESLICES = [(0, 512), (512, 256)]

f32 = mybir.dt.float32
f32r = mybir.dt.float32r
bf16 = mybir.dt.bfloat16
EXP = mybir.ActivationFunctionType.Exp
LN = mybir.ActivationFunctionType.Ln

_NC_CACHE = {}


def build(mm_dtype="bf16", e_dtype="bf16", crossbase=False):
    mdt = {"f32r": f32r, "f32": f32, "bf16": bf16}[mm_dtype]
    edt = {"f32r": f32r, "f32": f32, "bf16": bf16}[e_dtype]
    # The host pre-lays-out every tensor in its SBUF tile geometry (and
    # pre-casts to bf16 in bf16 mode), so every DMA is a contiguous burst:
    #   x  -> x^T as [P, KT, S]   (kills the on-device transpose entirely)
    #   Wq/Wk -> [NPAIR, P, KT, P] head-pair-major
    #   Wv/Wo -> [P, KT, E]
    #   bq/bk -> [P, KT]
    wdt = bf16 if mm_dtype == "bf16" else f32
    nc = bass.Bass()
    x_d = nc.declare_dram_parameter("x", [P, KT, S], wdt, isOutput=False)
    Wq_d = nc.declare_dram_parameter("Wq", [NPAIR, P, KT, P], wdt, isOutput=False)
    Wk_d = nc.declare_dram_parameter("Wk", [NPAIR, P, KT, P], wdt, isOutput=False)
    Wv_d = nc.declare_dram_parameter("Wv", [P, KT, E], wdt, isOutput=False)
    Wo_d = nc.declare_dram_parameter("Wo", [P, KT, E], wdt, isOutput=False)
    bq_d = nc.declare_dram_parameter("bq", [P, KT], f32, isOutput=False)
    bk_d = nc.declare_dram_parameter("bk", [P, KT], f32, isOutput=False)
    bv_d = nc.declare_dram_parameter("bv", [E], f32, isOutput=False)
    bo_d = nc.declare_dram_parameter("bo", [E], f32, isOutput=False)
    # bf16 output (host casts back to f32): halves the output DMA traffic
    out_d = nc.declare_dram_parameter("out", [S, E], wdt, isOutput=True)

    with ExitStack() as ctx:
        tc = ctx.enter_context(tile.TileContext(nc))
        singles = ctx.enter_context(tc.tile_pool(name="singles", bufs=1))
        xld = ctx.enter_context(tc.tile_pool(name="xld", bufs=2))
        wqk = ctx.enter_context(tc.tile_pool(name="wqk", bufs=2))
        wbig = ctx.enter_context(tc.tile_pool(name="wbig", bufs=1))
        qkp = ctx.enter_context(tc.tile_pool(name="qkp", bufs=2))
        ep = ctx.enter_context(tc.tile_pool(name="ep", bufs=2))
        np_pool = ctx.enter_context(tc.tile_pool(name="norm", bufs=2))
        outp = ctx.enter_context(tc.tile_pool(name="outp", bufs=2))
        # PSUM: mm(2) + S(2x2) + att(2) = 8 banks
        psum = ctx.enter_context(tc.tile_pool(name="psum", bufs=2, space="PSUM"))

        # ---- persistent big buffers ----
        xT = singles.tile([P, KT, S], mdt)          # x^T  [e_in, s]
        V_ext = singles.tile([P, ST, H, P], edt)    # [s, ktile, head, ...]
        concatT = singles.tile([P, NPAIR, S], mdt)  # attn^T by pair

        # ---- phase 1: load x^T (emitted first: it gates everything) ----
        # Three hardware DMA queues in parallel: x^T halves on the sync
        # queue, the pair-0-critical weights (Wv, Wq0/Wk0) on the scalar
        # HWDGE queue (ACT is idle early), the rest on the gpsimd queue.
        def xt_load(dst, src, eng):
            if mdt != f32r:
                eng.dma_start(dst, src)
            else:
                stg = xld.tile([P, S], f32, tag="x", name="xstage")
                nc.sync.dma_start(stg[:], src.rearrange("p k s -> p (k s)"))
                nc.vector.tensor_copy(
                    dst, stg[:].rearrange("p (k s) -> p k s", s=S))

        # three chunks on three queues in parallel (scalar carries wq0/wk0
        # first — they gate the first matmul — then its xT chunk)
        xt_load(xT[:, 0:2, :], x_d[:, 0:2, :], nc.sync)
        xt_load(xT[:, 2:4, :], x_d[:, 2:4, :], nc.gpsimd)

        def wload(dst_r, src_slice_ap, eng=None):
            """Load weights into an SBUF tile of matmul dtype.

            bf16/f32 mode: DRAM layout already matches, one direct DMA on
            the given DMA queue (default gpsimd).
            f32r mode: DMA f32 to a staging tile, DVE-cast into fp32r dst.
            """
            if mdt != f32r:
                (eng or nc.gpsimd).dma_start(dst_r[:], src_slice_ap)
            else:
                stg = wqk.tile(list(dst_r.shape), f32, tag="wstg", name="wstg")
                nc.sync.dma_start(stg[:], src_slice_ap)
                nc.vector.tensor_copy(dst_r[:], stg[:])

        # ---- constants ----
        bq_sb = singles.tile([P, KT], f32)
        bk_sb = singles.tile([P, KT], f32)
        nc.sync.dma_start(bq_sb[:], bq_d[:])
        nc.sync.dma_start(bk_sb[:], bk_d[:])

        def bcast_load(dst, src_ap):  # [E] -> [P, E] partition-step-0 DMA
            nc.gpsimd.dma_start(
                out=dst,
                in_=bass.AP(tensor=src_ap.tensor, offset=src_ap.offset,
                            ap=[[0, P]] + [list(a) for a in src_ap.ap]))
        bcast = ctx.enter_context(tc.tile_pool(name="bcast", bufs=1))
        bv_bc = bcast.tile([P, E], f32, tag="bbc")
        bcast_load(bv_bc[:], bv_d[:])

        # Only the ones-halves need the memset (the V projection writes the
        # V halves); split across DVE+GPSIMD so it clears in ~3us.
        V5 = V_ext[:].rearrange("p s (j par) c -> p s j par c", par=2)
        nc.vector.memset(V5[:, :, :, 0, D:P], 1.0)
        nc.gpsimd.memset(V5[:, :, :, 1, 0:D], 1.0)

        # ---- phase 2: V projection into V_ext ----
        # Emitted between the first scores and first attnV, so these matmuls
        # fill the PE while ACT computes the first exps.  PSUM evacuation is
        # batched per parity (strided APs over 4/2 heads at once) so the two
        # mm PSUM banks recycle fast enough to keep the PE streaming.
        def v_proj_all():
            for st in range(ST):
                Vx = V_ext[:, st, :, :].rearrange("p (j par) c -> p j par c",
                                                  par=2)
                for nsi, (noff, nsz) in enumerate(ESLICES):
                    pv = psum.tile([P, 512], f32, tag="mm")
                    for k in range(KT):
                        nc.tensor.matmul(
                            pv[:, :nsz],
                            xT[:, k, st * P:(st + 1) * P],
                            Wv_sb[:, k, noff:noff + nsz],
                            start=(k == 0), stop=(k == KT - 1),
                        )
                    j0, nj = noff // P, nsz // P
                    pv_v = pv[:, :nsz].rearrange("p (j par d) -> p j par d",
                                                 par=2, d=D)
                    bv_v = bv_bc[:, noff:noff + nsz].rearrange(
                        "p (j par d) -> p j par d", par=2, d=D)
                    nc.vector.tensor_add(Vx[:, j0:j0 + nj, 0, 0:D],
                                         pv_v[:, :, 0, :], bv_v[:, :, 0, :])
                    nc.vector.tensor_add(Vx[:, j0:j0 + nj, 1, D:P],
                                         pv_v[:, :, 1, :], bv_v[:, :, 1, :])

        # ---- phase 3: head pairs, software-pipelined ----
        # PE order per (pair, q-half): scores_m -> next pair's Q or K
        # projection matmuls -> attnV_m.  The projection matmuls fill the PE
        # while ACT computes this iteration's exps, keeping the PE dense (HAM
        # stays at full clock) instead of stalling on E.  The projections'
        # DVE bias-adds are deferred until after attnV's PSUM evacuation so
        # the att banks recycle first in the DVE queue.
        wq_t, wk_t, qt_t, kt_t = {}, {}, {}, {}
        def load_w(m, eng=None):
            wq_t[m] = wqk.tile([P, KT, P], mdt, tag="wq", name="wq_m")
            wk_t[m] = wqk.tile([P, KT, P], mdt, tag="wk", name="wk_m")
            wload(wq_t[m], Wq_d[m], eng=eng)
            wload(wk_t[m], Wk_d[m], eng=eng)

        # Scalar-queue order: pair-0 weights first (they gate the first
        # matmul), then Wv (first needed ~10us in).  Pair-1 and Wo/bo go on
        # the gpsimd queue.
        load_w(0, eng=nc.scalar)
        xt_load(xT[:, 4:6, :], x_d[:, 4:6, :], nc.scalar)
        Wv_sb = wbig.tile([P, KT, E], mdt, tag="wbig")
        wload(Wv_sb, Wv_d[:], eng=nc.scalar)
        load_w(1)
        bo_bc = bcast.tile([P, E], f32, tag="bbc")
        bcast_load(bo_bc[:], bo_d[:])
        Wo_sb = wbig.tile([P, KT, E], mdt, tag="wbig")
        wload(Wo_sb, Wo_d[:])

        def proj_mm(m, which, q2):
            """6 matmuls: one q-half of QT_m (or KT_m) into a PSUM tile."""
            w = wq_t[m] if which == "q" else wk_t[m]
            tmap = qt_t if which == "q" else kt_t
            if m not in tmap:
                tmap[m] = qkp.tile([P, S], mdt, tag=which + "t", name=which + "t")
            qsl = slice(q2 * QTILE, (q2 + 1) * QTILE)
            pq = psum.tile([P, 512], f32, tag="mm", name="pq")
            for k in range(KT):
                nc.tensor.matmul(pq[:], w[:, k, :], xT[:, k, qsl],
                                 start=(k == 0), stop=(k == KT - 1))
            return pq

        def proj_fin(m, which, q2, pq):
            """Deferred DVE bias-add: PSUM -> QT/KT tile."""
            bias = bq_sb if which == "q" else bk_sb
            t = (qt_t if which == "q" else kt_t)[m]
            qsl = slice(q2 * QTILE, (q2 + 1) * QTILE)
            nc.vector.tensor_scalar_add(t[:, qsl], pq[:], bias[:, m:m + 1])

        def proj_half(m, which, q2):
            proj_fin(m, which, q2, proj_mm(m, which, q2))

        def scores_exp(m, q2):
            """Score matmuls + exp for both heads of pair m, one q-half."""
            qt_m, kt_m = qt_t[m], kt_t[m]
            qsl = slice(q2 * QTILE, (q2 + 1) * QTILE)
            e_a = ep.tile([P, ST, QTILE], edt, tag="eA")
            e_b = ep.tile([P, ST, QTILE], edt, tag="eB")
            for c in range(ST // 2):
                s_a = psum.tile([P, 2, 512], f32, tag="S")
                s_b = psum.tile([P, 2, 512], f32, tag="S")
                for kk in range(2):
                    ktile = c * 2 + kk
                    ksl = slice(ktile * P, (ktile + 1) * P)
                    nc.tensor.matmul(s_a[:, kk, :], kt_m[0:D, ksl],
                                     qt_m[0:D, qsl], start=True, stop=True)
                    nc.tensor.matmul(s_b[:, kk, :], kt_m[D:P, ksl],
                                     qt_m[D:P, qsl], start=True, stop=True)
                nc.scalar.activation(e_a[:, c * 2:c * 2 + 2, :], s_a[:], EXP, scale=0.125)
                nc.scalar.activation(e_b[:, c * 2:c * 2 + 2, :], s_b[:], EXP, scale=0.125)
            return e_a, e_b

        def attnv_norm(m, q2, e_a, e_b):
            """attnV matmuls + softmax normalization, one q-half."""
            qsl = slice(q2 * QTILE, (q2 + 1) * QTILE)
            # attnV: rows [attn|sums] (even head) / [sums|attn] (odd head)
            p_a = psum.tile([P, 512], f32, tag="att")
            p_b = psum.tile([P, 512], f32, tag="att")
            for ktile in range(ST):
                nc.tensor.matmul(p_a[:], V_ext[:, ktile, 2 * m, :],
                                 e_a[:, ktile, :],
                                 start=(ktile == 0), stop=(ktile == ST - 1))
            for ktile in range(ST):
                nc.tensor.matmul(p_b[:], V_ext[:, ktile, 2 * m + 1, :],
                                 e_b[:, ktile, :],
                                 start=(ktile == 0), stop=(ktile == ST - 1))
            # sums half-swap: partition-aligned DVE copies (crossbase DVE
            # PSUM reads are broken, see module docstring) + SBUF DMA; the
            # reciprocal is exp(-ln(sums)) on ACT (~3x faster than DVE
            # InstReciprocal), then DVE multiplies straight out of the att
            # banks into concatT.
            sums_t = np_pool.tile([P, 512], f32, tag="sums_t")
            attv = np_pool.tile([P, 512], f32, tag="attv")
            nc.vector.tensor_copy(sums_t[D:P, :], p_a[D:P, :])
            nc.vector.tensor_copy(attv[0:D, :], p_a[0:D, :])
            nc.vector.tensor_copy(sums_t[0:D, :], p_b[0:D, :])
            nc.vector.tensor_copy(attv[D:P, :], p_b[D:P, :])
            sums = np_pool.tile([P, 512], f32, tag="sums")
            nc.sync.dma_start(sums[0:D, :], sums_t[D:P, :])
            nc.sync.dma_start(sums[D:P, :], sums_t[0:D, :])
            return sums, attv

        def norm_fin(m, q2, sums, attv):
            # 1/sums as exp(-ln(sums)) on ACT: ~3x faster than the DVE
            # InstReciprocal and keeps the DVE queue free for evacuations.
            # Pair 0 uses the DVE reciprocal instead: during warmup ACT is
            # the pipeline-fill bottleneck (first exps) while DVE has slack.
            qsl = slice(q2 * QTILE, (q2 + 1) * QTILE)
            if m == 0:
                nc.vector.reciprocal(sums[:], sums[:])
            else:
                lns = np_pool.tile([P, 512], f32, tag="lns")
                nc.scalar.activation(lns[:], sums[:], LN)
                nc.scalar.activation(sums[:], lns[:], EXP, scale=-1.0)
            nc.vector.tensor_mul(concatT[:, m, qsl], attv[:], sums[:])

        def out_proj(st, k_hi=KT, pos=None, tag="mm"):
            """Output projection for one s-tile.

            k_hi < KT emits a partial accumulation (pairs 0..k_hi-1) and
            returns the open PSUM groups; call again with pos=... to add the
            remaining pairs, close the groups, and store.  tag="S" borrows
            the (free, post-scores) S-tile banks for extra partials.
            """
            pos = pos or {}
            k_lo = pos.pop("k_lo", 0)
            for nsi, (noff, nsz) in enumerate(ESLICES):
                po = pos.get(nsi)
                if po is None:
                    if tag == "S":
                        s_po = psum.tile([P, 2, 512], f32, tag="S", name="po")
                        po = s_po[:, nsi, :]
                        if nsi == 0:
                            pos["s_tile"] = s_po
                    else:
                        po = psum.tile([P, 512], f32, tag="mm", name="po")[:]
                    pos[nsi] = po
                for k in range(k_lo, k_hi):
                    nc.tensor.matmul(
                        po[:, :nsz],
                        concatT[:, k, st * P:(st + 1) * P],
                        Wo_sb[:, k, noff:noff + nsz],
                        start=(k == 0), stop=(k == KT - 1),
                    )
            if k_hi < KT:
                pos["k_lo"] = k_hi
                return pos
            o_sb = outp.tile([P, E], wdt, tag="o")
            oq = nc.sync if st % 2 == 0 else nc.scalar
            for nsi, (noff, nsz) in enumerate(ESLICES):
                po = pos[nsi]
                nc.vector.tensor_add(o_sb[:, noff:noff + nsz], po[:, :nsz],
                                     bo_bc[:, noff:noff + nsz])
                oq.dma_start(out_d[st * P:(st + 1) * P, noff:noff + nsz],
                             o_sb[:, noff:noff + nsz])
            return None

        # Priming: only the pair-0 projections the first scores needs (qt
        # half-1 is deferred past them), first scores, then qt half-1 and
        # the whole V projection as the PE filler under the first exps.
        proj_half(0, "q", 0)
        proj_half(0, "k", 0)
        proj_half(0, "k", 1)
        e_pend = scores_exp(0, 0)
        proj_half(0, "q", 1)
        v_proj_all()
        for m in range(NPAIR):
            last = m + 1 == NPAIR
            if m >= 1 and not last:
                load_w(m + 1)
            for q2 in range(NQ):
                ea, eb = e_pend if (m, q2) == (0, 0) else scores_exp(m, q2)
                # Fill the PE while ACT computes this iteration's exps:
                # next pair's projection matmuls (bias-adds deferred so the
                # attnV evacuation copies go first in the DVE queue), or on
                # the last pair the first half of the output projection,
                # which only needs q2=0 of concatT.  Slot (0,0) is filled by
                # the V projection above; slot (0,1) carries pair-1's q and
                # k projections (q with inline fins — only two mm PSUM bufs).
                pqs = []
                if (m, q2) == (0, 0):
                    pass
                elif (m, q2) == (0, 1):
                    proj_half(1, "q", 0)
                    proj_half(1, "q", 1)
                    pqs = [("k", h2, proj_mm(1, "k", h2)) for h2 in range(NQ)]
                elif not last:
                    wh = "q" if q2 == 0 else "k"
                    pqs = [(wh, h2, proj_mm(m + 1, wh, h2)) for h2 in range(NQ)]
                elif q2 == 1:
                    for st in range(ST // 2):
                        out_proj(st)
                sums, attv = attnv_norm(m, q2, ea, eb)
                if last and q2 == 1:
                    # fill the norm-chain wait: st 4..6 over pairs 0..4
                    # (pair 5 accumulates in the finishers below once its
                    # concatT lands); st 5/6 borrow the freed S banks
                    parts = [out_proj(4, k_hi=KT - 1),
                             out_proj(5, k_hi=KT - 1, tag="S"),
                             out_proj(6, k_hi=KT - 1, tag="S")]
                for wh, h2, pq in pqs:
                    proj_fin(m + 1, wh, h2, pq)
                norm_fin(m, q2, sums, attv)
        # ---- phase 4: output projection (second half) ----
        for st, pos in zip(range(4, ST), parts + [None]):
            out_proj(st, pos=pos)

    _split_excess_waits(nc)
    return nc


def run_spmd(inputs, Wq, bq, Wk, bk, Wv, bv, Wo, bo,
             mm_dtype="bf16", e_dtype="bf16", crossbase=False, trace=False):
    key = (mm_dtype, e_dtype, crossbase)
    if key not in _NC_CACHE:
        _NC_CACHE[key] = build(mm_dtype, e_dtype, crossbase)
    nc = _NC_CACHE[key]
    if mm_dtype == "bf16":
        import ml_dtypes
        wnp = ml_dtypes.bfloat16
    else:
        wnp = np.float32
    # Host-side layout prep: every tensor lands in its SBUF tile geometry so
    # every device DMA is a contiguous burst (see build()).
    x = np.asarray(inputs, dtype=np.float32)
    Wq_h = (np.asarray(Wq, np.float32).reshape(KT, P, NPAIR, P)
            .transpose(2, 1, 0, 3).astype(wnp))
    Wk_h = (np.asarray(Wk, np.float32).reshape(KT, P, NPAIR, P)
            .transpose(2, 1, 0, 3).astype(wnp))
    Wv_h = (np.asarray(Wv, np.float32).reshape(KT, P, E)
            .transpose(1, 0, 2).astype(wnp))
    Wo_h = (np.asarray(Wo, np.float32).reshape(KT, P, E)
            .transpose(1, 0, 2).astype(wnp))
    common = {
        "Wq": Wq_h, "Wk": Wk_h, "Wv": Wv_h, "Wo": Wo_h,
        "bq": np.ascontiguousarray(np.asarray(bq, np.float32).reshape(KT, P).T),
        "bk": np.ascontiguousarray(np.asarray(bk, np.float32).reshape(KT, P).T),
        "bv": np.asarray(bv, np.float32), "bo": np.asarray(bo, np.float32),
    }
    in_maps = [
        dict(common,
             x=x[b].T.reshape(KT, P, S).transpose(1, 0, 2).astype(wnp))
        for b in range(x.shape[0])
    ]
    res = run_bass_kernel_spmd(nc, in_maps, core_ids=list(range(len(in_maps))),
                               trace=trace)
    out = np.stack([res.results[b]["out"] for b in range(len(in_maps))],
                   axis=0).astype(np.float32)
    return out, res


def kernel(inputs, Wq, bq, Wk, bk, Wv, bv, Wo, bo):
    out, _ = run_spmd(inputs, Wq, bq, Wk, bk, Wv, bv, Wo, bo)
    return out

